# revision 26
# baseline (speedup 1.0000x reference)
"""Trainium2 Bass kernel for nn_BaselineTargetHead (per-sample dynamic MLP).

Strategy: data-parallel over 8 NeuronCores, 8 samples per core.
Per sample the chain is 5 per-sample linear layers over 64 spatial positions:
  [1024,2048] @ [2048,64] -> sigmoid -> ... -> [1,128] @ [128,64] + b

fc1-fc4 weights (99.9% of bytes) and the input x ship as fp8 e3m4 (4
mantissa bits). Host pre-scales weights by 64 (x by 2) to center N(0,0.02)
data in e3m4's normal range; the inverse scale folds into the ScalarE
activation's `scale`. fc5 weights stay fp16: the output is a 128-term dot
product with no downstream averaging, so fc5 quantization dominates the
error budget (quantizing w5 alone costs 1.4e-2 rel err; w1-w4 cost ~1e-3).

The kernel sits at the ridge: Tensor ~61 us busy (45 ns per
LDWEIGHTS+MATMUL pair, 171 pairs/sample) vs DMA ~61-68 us on a single
queue (the two HWDGE queues share ~370 B/ns of fabric, so splitting the
stream gains nothing). Scheduling details that matter:
  - everything lives in SBUF simultaneously (~186 KB/partition), so all
    DMAs are issued upfront with no tile rotation or flow-control stalls.
  - weight DMAs are TYPED fp16 and bitcast to fp8 at the matmul: the DMA
    engine moves ~10% faster with 2-byte elements (368 vs 334 B/ns
    measured on identical shapes/bytes).
  - fc1 is laid out m-major (col = m*2048 + k*128) and shipped in two
    chunks, so each chunk enables complete m-tiles immediately — the last
    sample's fc1 compute overlaps its own DMA tail.
  - the previous sample's tiny fc4/fc5 are interleaved into fc1's m-groups
    so their input activations (314 ns ScalarE latency each) resolve
    behind ~2.7 us of fc1 matmuls instead of stalling the PE (~1.4
    us/sample of layer-boundary gaps otherwise).
  - matmul: lhsT = W^T tile [128(Cin), 128(Cout)] fp8 (FWL halves the
    weight-load time), rhs = activation tile [128(Cin), 64(spatial)] fp16,
    accumulated over Cin tiles in PSUM fp32. ScalarE applies
    scale+bias+sigmoid fused, writing fp16 tiles that feed the next layer
    without any transposition.
"""

import numpy as np
import ml_dtypes

import concourse.bass as bass
import concourse.mybir as mybir
import concourse.tile as tile
from concourse.bass_utils import run_bass_kernel_spmd

N_CORES = 8
B = 64
S_PER_CORE = B // N_CORES  # 8 samples per core
HW = 64  # 8x8 spatial positions
LAYERS = [(2048, 1024), (1024, 512), (512, 256), (256, 128)]  # (Cin, Cout) of fc1..fc4
W_SCALE_FP8 = 64.0  # host multiplies fp8 weights by this; kernel divides back
X_SCALE_FP8 = 2.0  # same for the input x image
A_COLS = (LAYERS[0][0] // 128) * LAYERS[0][1]  # 16384 fp8 cols (fc1, m-major)
B_COLS = sum((ci // 128) * co for ci, co in LAYERS[1:])  # 5376 fp8 cols (fc2-4)
X_COLS = (2048 // 128) * HW  # 1024
W5_COLS = 32  # w5 zero-padded to 32 cols for a legal M=32 matmul
# bias image columns per sample: fc1 m0..7 | fc2 m0..3 | fc3 m0..1 | fc4 m0 | fc5
BIAS_COL0 = [0, 8, 12, 14]
BIAS_COLS = 16
# per-layer PSUM scale to undo the host-side fp8 pre-scaling
ACT_SCALE = [
    1.0 / (W_SCALE_FP8 * X_SCALE_FP8),
    1.0 / W_SCALE_FP8,
    1.0 / W_SCALE_FP8,
    1.0 / W_SCALE_FP8,
]
# per-layer base fp8 column of each layer's weights within a sample's B image
B_OFF = [0, 4096, 4096 + 1024]  # fc2, fc3, fc4


def _split_ctrl_multiwaits(nc):
    """walrus in this env rejects >1 sync-wait per instruction. Move extra
    waits onto NOPs placed immediately before, on the same engine — engines
    execute in order, so this is semantically identical."""
    n_fixed = 0
    for bb in nc.main_func.blocks:
        insts = bb.instructions
        i = 0
        while i < len(insts):
            ins = insts[i]
            si = ins.sync_info
            if si is not None and si.on_wait and len(si.on_wait) > 1:
                waits = list(si.on_wait)
                new_nops = []
                for j, w in enumerate(waits[1:]):
                    nop = mybir.InstNoOp(name=f"{ins.name}-splitw-{j}", ins=[], outs=[])
                    nop.engine = ins.engine
                    nop.sync_info = mybir.SyncInfo(on_update=[], on_wait=[w])
                    new_nops.append(nop)
                si.on_wait = [waits[0]]
                insts[i:i] = new_nops
                i += len(new_nops)
                n_fixed += 1
            i += 1
    return n_fixed


def _build_nc():
    f8 = mybir.dt.float8e3
    f16 = mybir.dt.float16
    f32 = mybir.dt.float32
    nc = bass.Bass()
    # weight/x images carry fp8 bytes but are typed fp16 for the DMA (2-byte
    # elements stream faster); compute slices bitcast back to fp8. Each
    # sample's x rides at the head of its own slab so the whole sample is
    # one maximal-row-length transfer: [x | fc1 m-major | fc2-4].
    SLAB_F16 = (X_COLS + A_COLS + B_COLS) // 2  # 11392
    # two samples share one DRAM row (45.6 KB contiguous runs): the DMA
    # fabric rate rises with run length (330 B/ns @ 8-16 KB, 351 @ 22.8 KB)
    wslab_d = nc.dram_tensor(
        "wslab", [S_PER_CORE // 2, 128, 2 * SLAB_F16], f16, kind="ExternalInput"
    )
    w5img_d = nc.dram_tensor("w5img", [128, S_PER_CORE * W5_COLS], f16, kind="ExternalInput")
    bias_d = nc.dram_tensor("bias", [128, S_PER_CORE * BIAS_COLS], f32, kind="ExternalInput")
    out_d = nc.dram_tensor("out", [1, S_PER_CORE * HW], f32, kind="ExternalOutput")

    sig = mybir.ActivationFunctionType.Sigmoid
    ident = mybir.ActivationFunctionType.Identity

    with tile.TileContext(nc) as tc:
        with (
            tc.tile_pool(name="wpool", bufs=1) as wpool,
            tc.tile_pool(name="qpool", bufs=3) as qpool,
            tc.tile_pool(name="psum", bufs=8, space="PSUM") as psum_pool,
        ):
            # ---- all DMAs issued upfront; everything fits in SBUF ----
            # A single SP-queue stream (concurrent queues derate the shared
            # DMA fabric: 2x~140 B/ns vs ~330 solo). One transfer per sample
            # maximizes row length; sample 0 is chunked so the PE starts
            # early, the last sample so its fc1 m-groups land incrementally
            # (fc2-4 early) and compute overlaps the stream tail.
            bias_sb = wpool.tile([128, S_PER_CORE * BIAS_COLS], f32)
            nc.scalar.dma_start(bias_sb[:], bias_d[:])
            w5_sb = wpool.tile([128, S_PER_CORE * W5_COLS], f16)
            nc.scalar.dma_start(w5_sb[:], w5img_d[:])
            XF = X_COLS // 2  # 512 f16 cols of x per sample
            HAF = A_COLS // 4  # 4096 f16 cols per fc1 m-group half
            wpair, wa_view = [], []
            last = S_PER_CORE - 1
            for p in range(S_PER_CORE // 2):
                tp = wpool.tile([128, 2 * SLAB_F16], f16, name=f"wp{p}")
                wpair.append(tp)
                wa_view.append((tp, 0))
                wa_view.append((tp, SLAB_F16))
                if p == 0:
                    # sample 0 fine-chunked so the PE starts early
                    cuts = [0, XF + HAF // 2, XF + HAF, XF + 2 * HAF, SLAB_F16, 2 * SLAB_F16]
                elif p == S_PER_CORE // 2 - 1:
                    # sample 7 fine-chunked, fc2-4 early, so its fc1 m-groups
                    # land incrementally and compute overlaps the stream tail
                    L = SLAB_F16
                    for a, b in [
                        (0, L),  # sample 6 whole
                        (L, L + XF + HAF // 2),  # s7: x + fc1 m0-1
                        (L + XF + 2 * HAF, 2 * L),  # s7: fc2-4
                        (L + XF + HAF // 2, L + XF + HAF),  # s7: fc1 m2-3
                        (L + XF + HAF, L + XF + HAF + HAF // 2),  # s7: fc1 m4-5
                        (L + XF + HAF + HAF // 2, L + XF + 2 * HAF),  # s7: fc1 m6-7
                    ]:
                        nc.sync.dma_start(tp[:, a:b], wslab_d[p, :, a:b])
                    continue
                else:
                    cuts = [0, 2 * SLAB_F16]
                for a, b in zip(cuts[:-1], cuts[1:]):
                    nc.sync.dma_start(tp[:, a:b], wslab_d[p, :, a:b])
            # all samples' outputs land in partition 0 of one tile
            # (sample s -> columns s*HW..(s+1)*HW) so one DMA ships them all
            ot_all = wpool.tile([128, S_PER_CORE * HW], f32)

            # ---- compute ----
            def w_slice(s, li, k, m):
                """fp8 lhsT [128, 128] for (layer, k-tile, m-tile) of sample s."""
                if li == 0:
                    col = X_COLS + m * 2048 + k * 128  # m-major fc1 after x
                else:
                    col = X_COLS + A_COLS + B_OFF[li - 1] + k * LAYERS[li][1] + m * 128
                t, base = wa_view[s]
                c = base + col // 2
                return t[:, c : c + 64].bitcast(mybir.dt.float8e3)

            def x_slice(s, k):
                t, base = wa_view[s]
                c = base + k * HW // 2
                return t[:, c : c + HW // 2].bitcast(mybir.dt.float8e3)

            q_tiles = [None] * S_PER_CORE  # per-sample [q1, q2, q3, q4]

            def emit_layer(s, li, m_range, q_prev_fn):
                cin, cout = LAYERS[li]
                kt = cin // 128
                qn = q_tiles[s][li]
                for m in m_range:
                    ps = psum_pool.tile([128, HW], f32, tag="ps")
                    for k in range(kt):
                        nc.tensor.matmul(
                            ps[:],
                            w_slice(s, li, k, m),
                            q_prev_fn(k),
                            start=(k == 0),
                            stop=(k == kt - 1),
                        )
                    bcol = s * BIAS_COLS + BIAS_COL0[li] + m
                    nc.scalar.activation(
                        qn[:, m * HW : (m + 1) * HW],
                        ps[:],
                        sig,
                        bias=bias_sb[:, bcol : bcol + 1],
                        scale=ACT_SCALE[li],
                    )

            def emit_fc5(s):
                ps5 = psum_pool.tile([128, HW], f32, tag="ps", name=f"ps5_{s}")
                w5t = w5_sb[:, s * W5_COLS : (s + 1) * W5_COLS]
                nc.tensor.matmul(
                    ps5[0:32, :], w5t, q_tiles[s][3][:, 0:HW], start=True, stop=True
                )
                b5col = s * BIAS_COLS + 15
                nc.scalar.activation(
                    ot_all[0:1, s * HW : (s + 1) * HW],
                    ps5[0:1, :],
                    ident,
                    bias=bias_sb[0:1, b5col : b5col + 1],
                    scale=1.0,
                )
                nc.scalar.dma_start(
                    out_d[0:1, s * HW : (s + 1) * HW],
                    ot_all[0:1, s * HW : (s + 1) * HW],
                )

            for s in range(S_PER_CORE):
                q_tiles[s] = [
                    qpool.tile(
                        [128, (LAYERS[li][1] // 128) * HW],
                        f16,
                        tag=f"q{li}",
                        name=f"q{li}_{s}",
                    )
                    for li in range(4)
                ]
                xf = lambda k, s=s: x_slice(s, k)
                # fc1 m0-3; the previous sample's fc4 resolves its fc3
                # activations behind these 2.7 us of matmuls
                emit_layer(s, 0, range(0, 4), xf)
                if s > 0:
                    emit_layer(s - 1, 3, range(0, 1), lambda k, p=s - 1: q_tiles[p][2][:, k * HW : (k + 1) * HW])
                emit_layer(s, 0, range(4, 8), xf)
                if s > 0:
                    emit_fc5(s - 1)
                emit_layer(s, 1, range(0, 4), lambda k, s=s: q_tiles[s][0][:, k * HW : (k + 1) * HW])
                emit_layer(s, 2, range(0, 2), lambda k, s=s: q_tiles[s][1][:, k * HW : (k + 1) * HW])
            last = S_PER_CORE - 1
            emit_layer(last, 3, range(0, 1), lambda k: q_tiles[last][2][:, k * HW : (k + 1) * HW])
            emit_fc5(last)

    _split_ctrl_multiwaits(nc)
    return nc


_NC_CACHE = None


def _get_nc():
    global _NC_CACHE
    if _NC_CACHE is None:
        _NC_CACHE = _build_nc()
    return _NC_CACHE


def _to_e3m4(a, scale):
    return np.clip(a * scale, -14.0, 14.0).astype(ml_dtypes.float8_e3m4)


def _prep_core(inputs, c):
    """Build the per-core input map (numpy only, host-side layout prep)."""
    sl = slice(c * S_PER_CORE, (c + 1) * S_PER_CORE)

    def wimg(li):
        cin, cout = LAYERS[li]
        w = inputs[f"target_fc{li + 1}w"][sl, :, :, 0, 0]  # [S, Cout, Cin]
        # -> [S, 128, (Cin/128)*Cout] with img[s, p, k*Cout+co] = w[s, co, k*128+p]
        wt = w.transpose(0, 2, 1).reshape(S_PER_CORE, cin // 128, 128, cout)
        return wt.transpose(0, 2, 1, 3).reshape(S_PER_CORE, 128, -1)

    w1 = wimg(0)  # [S, 128, 16384] k-major: col = k*1024 + m*128
    # fc1 -> m-major: col = m*2048 + k*128
    w1 = (
        w1.reshape(S_PER_CORE, 128, 16, 8, 128)
        .transpose(0, 1, 3, 2, 4)
        .reshape(S_PER_CORE, 128, A_COLS)
    )
    x = inputs["target_in_vec"][sl].reshape(S_PER_CORE, 2048 // 128, 128, HW)
    ximg = _to_e3m4(x.transpose(0, 2, 1, 3).reshape(S_PER_CORE, 128, X_COLS), X_SCALE_FP8)
    wq = _to_e3m4(
        np.concatenate([w1] + [wimg(li) for li in (1, 2, 3)], axis=2), W_SCALE_FP8
    )
    wslab = (
        np.ascontiguousarray(
            np.concatenate([ximg, wq], axis=2)
            .reshape(S_PER_CORE // 2, 2, 128, -1)
            .transpose(0, 2, 1, 3)
            .reshape(S_PER_CORE // 2, 128, -1)
        )
        .view(np.uint8)
        .reshape(S_PER_CORE // 2, 128, -1)
        .view(np.float16)
    )

    w5 = inputs["target_fc5w"][sl, 0, :, 0, 0].astype(np.float16)  # [S, 128]
    w5img = np.zeros((128, S_PER_CORE, W5_COLS), np.float16)
    w5img[:, :, 0] = w5.T
    w5img = np.ascontiguousarray(w5img.reshape(128, -1))

    bias = np.zeros((S_PER_CORE, 128, BIAS_COLS), np.float32)
    for li, (cin, cout) in enumerate(LAYERS):
        b = inputs[f"target_fc{li + 1}b"][sl]  # [S, Cout]
        bias[:, :, BIAS_COL0[li] : BIAS_COL0[li] + cout // 128] = b.reshape(
            S_PER_CORE, cout // 128, 128
        ).transpose(0, 2, 1)
    bias[:, 0, 15] = inputs["target_fc5b"][sl, 0]
    bias = np.ascontiguousarray(bias.transpose(1, 0, 2).reshape(128, -1))

    return {"wslab": wslab, "w5img": w5img, "bias": bias}


def kernel(**inputs):
    inputs = {k: np.asarray(v) for k, v in inputs.items()}
    nc = _get_nc()
    in_maps = [_prep_core(inputs, c) for c in range(N_CORES)]
    res = run_bass_kernel_spmd(nc, in_maps, list(range(N_CORES)))
    out = np.concatenate([np.asarray(res.results[c]["out"]) for c in range(N_CORES)], axis=0)
    return out.reshape(B, 8, 8).astype(np.float32)


# revision 27
# speedup vs baseline: 1.1176x; 1.1176x over previous
"""Trainium2 Bass kernel for nn_BaselineTargetHead (per-sample dynamic MLP).

Strategy: data-parallel over 8 NeuronCores, 8 samples per core.
Per sample the chain is 5 per-sample linear layers over 64 spatial positions:
  [1024,2048] @ [2048,64] -> sigmoid -> ... -> [1,128] @ [128,64] + b

fc1-fc4 weights (99.9% of bytes) and the input x ship as fp8 e3m4 (4
mantissa bits). Host pre-scales weights by 64 (x by 2) to center N(0,0.02)
data in e3m4's normal range; the inverse scale folds into the ScalarE
activation's `scale`. fc5 weights stay fp16: the output is a 128-term dot
product with no downstream averaging, so fc5 quantization dominates the
error budget (quantizing w5 alone costs 1.4e-2 rel err; w1-w4 cost ~1e-3).

The kernel sits at the ridge: Tensor ~55-61 us busy (40-47 ns per
LDWEIGHTS+MATMUL pair, 171 pairs/sample) vs DMA ~68 us on a single queue.
The 8 cores contend for chip HBM (~2.6-2.9 TB/s aggregate), so per-core
stream rate tops out around 330-350 B/ns; splitting across two HWDGE
queues (or gpsimd SWDGE) derates the shared fabric and loses. Scheduling
details that matter:
  - everything lives in SBUF simultaneously (~186 KB/partition), so all
    DMAs are issued upfront with no tile rotation. Bulk DMAs must NOT be
    issued from the ACT engine: the tile framework's per-queue flow
    control would block ScalarE (and all activations) behind transfers.
  - weight DMAs are typed fp16 and bitcast to fp8 at the matmul.
  - fc1 is laid out m-major (col = m*2048 + k*128) and shipped in two
    chunks (four for sample 0), so each chunk enables complete m-tiles
    immediately — the last sample's fc1 compute overlaps its own DMA tail
    and sample 0's first matmul fires ~4 us earlier.
  - the previous sample's tiny fc4/fc5 are interleaved into fc1's m-groups
    so their input activations (314 ns ScalarE latency each) resolve
    behind ~2.7 us of fc1 matmuls instead of stalling the PE (~1.4
    us/sample of layer-boundary gaps otherwise).
  - matmul: lhsT = W^T tile [128(Cin), 128(Cout)] fp8 (FWL halves the
    weight-load time), rhs = activation tile [128(Cin), 64(spatial)] fp16,
    accumulated over Cin tiles in PSUM fp32. ScalarE applies
    scale+bias+sigmoid fused, writing fp16 tiles that feed the next layer
    without any transposition.
"""

import numpy as np
import ml_dtypes

import concourse.bass as bass
import concourse.mybir as mybir
import concourse.tile as tile
from concourse.bass_utils import run_bass_kernel_spmd

N_CORES = 8
B = 64
S_PER_CORE = B // N_CORES  # 8 samples per core
HW = 64  # 8x8 spatial positions
LAYERS = [(2048, 1024), (1024, 512), (512, 256), (256, 128)]  # (Cin, Cout) of fc1..fc4
W_SCALE_FP8 = 64.0  # host multiplies fp8 weights by this; kernel divides back
X_SCALE_FP8 = 2.0  # same for the input x image
A_COLS = (LAYERS[0][0] // 128) * LAYERS[0][1]  # 16384 fp8 cols (fc1, m-major)
B_COLS = sum((ci // 128) * co for ci, co in LAYERS[1:])  # 5376 fp8 cols (fc2-4)
X_COLS = (2048 // 128) * HW  # 1024
W5_COLS = 32  # w5 zero-padded to 32 cols for a legal M=32 matmul
# bias image columns per sample: fc1 m0..7 | fc2 m0..3 | fc3 m0..1 | fc4 m0 | fc5
BIAS_COL0 = [0, 8, 12, 14]
BIAS_COLS = 16
# per-layer PSUM scale to undo the host-side fp8 pre-scaling
ACT_SCALE = [
    1.0 / (W_SCALE_FP8 * X_SCALE_FP8),
    1.0 / W_SCALE_FP8,
    1.0 / W_SCALE_FP8,
    1.0 / W_SCALE_FP8,
]
# per-layer base fp8 column of each layer's weights within a sample's B image
B_OFF = [0, 4096, 4096 + 1024]  # fc2, fc3, fc4


def _split_ctrl_multiwaits(nc):
    """walrus in this env rejects >1 sync-wait per instruction. Move extra
    waits onto NOPs placed immediately before, on the same engine — engines
    execute in order, so this is semantically identical."""
    n_fixed = 0
    for bb in nc.main_func.blocks:
        insts = bb.instructions
        i = 0
        while i < len(insts):
            ins = insts[i]
            si = ins.sync_info
            if si is not None and si.on_wait and len(si.on_wait) > 1:
                waits = list(si.on_wait)
                new_nops = []
                for j, w in enumerate(waits[1:]):
                    nop = mybir.InstNoOp(name=f"{ins.name}-splitw-{j}", ins=[], outs=[])
                    nop.engine = ins.engine
                    nop.sync_info = mybir.SyncInfo(on_update=[], on_wait=[w])
                    new_nops.append(nop)
                si.on_wait = [waits[0]]
                insts[i:i] = new_nops
                i += len(new_nops)
                n_fixed += 1
            i += 1
    return n_fixed


def _build_nc():
    f8 = mybir.dt.float8e3
    f16 = mybir.dt.float16
    f32 = mybir.dt.float32
    nc = bass.Bass()
    # weight/x images carry fp8 bytes but are typed fp16 for the DMA; compute
    # slices bitcast back to fp8.
    wslab_d = nc.dram_tensor(
        "wslab", [S_PER_CORE, 128, (A_COLS + B_COLS) // 2], f16, kind="ExternalInput"
    )
    ximg_d = nc.dram_tensor(
        "ximg", [128, S_PER_CORE * X_COLS // 2], f16, kind="ExternalInput"
    )
    w5img_d = nc.dram_tensor("w5img", [128, S_PER_CORE * W5_COLS], f16, kind="ExternalInput")
    bias_d = nc.dram_tensor("bias", [128, S_PER_CORE * BIAS_COLS], f32, kind="ExternalInput")
    out_d = nc.dram_tensor("out", [1, S_PER_CORE * HW], f32, kind="ExternalOutput")

    sig = mybir.ActivationFunctionType.Sigmoid
    ident = mybir.ActivationFunctionType.Identity

    with tile.TileContext(nc) as tc:
        with (
            tc.tile_pool(name="wpool", bufs=1) as wpool,
            tc.tile_pool(name="qpool", bufs=2) as qpool,
            tc.tile_pool(name="psum", bufs=6, space="PSUM") as psum_pool,
        ):
            # ---- all DMAs issued upfront; everything fits in SBUF ----
            # SP queue: sample 0's x slice first (it gates the first matmul),
            # then the weight slabs, sample-major: fc1 m0-3 | fc1 m4-7 | fc2-4
            # (sample 0's fc1 in four chunks so the PE starts sooner).
            x_sb = wpool.tile([128, S_PER_CORE * X_COLS // 2], f16)
            nc.sync.dma_start(x_sb[:, 0 : X_COLS // 2], ximg_d[:, 0 : X_COLS // 2])
            wa_sb, wb_sb = [], []
            for s in range(S_PER_CORE):
                ta = wpool.tile([128, A_COLS // 2], f16, name=f"wa{s}")
                nchunk = 4 if s == 0 else 2
                step = A_COLS // 2 // nchunk
                for j in range(nchunk):
                    nc.sync.dma_start(
                        ta[:, j * step : (j + 1) * step],
                        wslab_d[s, :, j * step : (j + 1) * step],
                    )
                wa_sb.append(ta)
                tb = wpool.tile([128, B_COLS // 2], f16, name=f"wb{s}")
                nc.sync.dma_start(
                    tb[:], wslab_d[s, :, A_COLS // 2 : (A_COLS + B_COLS) // 2]
                )
                wb_sb.append(tb)
            # ACT queue: rest of x, bias, w5 — all needed only after ~15 us,
            # and this queue drains early so per-sample output DMAs are
            # never stuck behind weight traffic.
            nc.scalar.dma_start(x_sb[:, X_COLS // 2 :], ximg_d[:, X_COLS // 2 :])
            bias_sb = wpool.tile([128, S_PER_CORE * BIAS_COLS], f32)
            nc.scalar.dma_start(bias_sb[:], bias_d[:])
            w5_sb = wpool.tile([128, S_PER_CORE * W5_COLS], f16)
            nc.scalar.dma_start(w5_sb[:], w5img_d[:])
            # all samples' outputs land in partition 0 of one tile
            # (sample s -> columns s*HW..(s+1)*HW) so one DMA ships them all
            ot_all = wpool.tile([128, S_PER_CORE * HW], f32)

            # ---- compute ----
            def w_slice(s, li, k, m):
                """fp8 lhsT [128, 128] for (layer, k-tile, m-tile) of sample s."""
                if li == 0:
                    col = m * 2048 + k * 128  # m-major fc1 layout
                    return wa_sb[s][:, col // 2 : col // 2 + 64].bitcast(
                        mybir.dt.float8e3
                    )
                col = B_OFF[li - 1] + k * LAYERS[li][1] + m * 128
                return wb_sb[s][:, col // 2 : col // 2 + 64].bitcast(mybir.dt.float8e3)

            def x_slice(s, k):
                c = s * X_COLS + k * HW
                return x_sb[:, c // 2 : c // 2 + HW // 2].bitcast(mybir.dt.float8e3)

            q_tiles = [None] * S_PER_CORE  # per-sample [q1, q2, q3, q4]

            def emit_layer(s, li, m_range, q_prev_fn):
                cin, cout = LAYERS[li]
                kt = cin // 128
                qn = q_tiles[s][li]
                for m in m_range:
                    ps = psum_pool.tile([128, HW], f32, tag="ps")
                    for k in range(kt):
                        nc.tensor.matmul(
                            ps[:],
                            w_slice(s, li, k, m),
                            q_prev_fn(k),
                            start=(k == 0),
                            stop=(k == kt - 1),
                        )
                    bcol = s * BIAS_COLS + BIAS_COL0[li] + m
                    nc.scalar.activation(
                        qn[:, m * HW : (m + 1) * HW],
                        ps[:],
                        sig,
                        bias=bias_sb[:, bcol : bcol + 1],
                        scale=ACT_SCALE[li],
                    )

            def emit_fc5(s):
                ps5 = psum_pool.tile([128, HW], f32, tag="ps", name=f"ps5_{s}")
                w5t = w5_sb[:, s * W5_COLS : (s + 1) * W5_COLS]
                nc.tensor.matmul(
                    ps5[0:32, :], w5t, q_tiles[s][3][:, 0:HW], start=True, stop=True
                )
                b5col = s * BIAS_COLS + 15
                nc.scalar.activation(
                    ot_all[0:1, s * HW : (s + 1) * HW],
                    ps5[0:1, :],
                    ident,
                    bias=bias_sb[0:1, b5col : b5col + 1],
                    scale=1.0,
                )
                nc.scalar.dma_start(
                    out_d[0:1, s * HW : (s + 1) * HW],
                    ot_all[0:1, s * HW : (s + 1) * HW],
                )

            for s in range(S_PER_CORE):
                q_tiles[s] = [
                    qpool.tile(
                        [128, (LAYERS[li][1] // 128) * HW],
                        f16,
                        tag=f"q{li}",
                        name=f"q{li}_{s}",
                    )
                    for li in range(4)
                ]
                xf = lambda k, s=s: x_slice(s, k)
                # fc1 m0-3; the previous sample's fc4 resolves its fc3
                # activations behind these 2.7 us of matmuls
                emit_layer(s, 0, range(0, 4), xf)
                if s > 0:
                    emit_layer(s - 1, 3, range(0, 1), lambda k, p=s - 1: q_tiles[p][2][:, k * HW : (k + 1) * HW])
                emit_layer(s, 0, range(4, 8), xf)
                if s > 0:
                    emit_fc5(s - 1)
                emit_layer(s, 1, range(0, 4), lambda k, s=s: q_tiles[s][0][:, k * HW : (k + 1) * HW])
                emit_layer(s, 2, range(0, 2), lambda k, s=s: q_tiles[s][1][:, k * HW : (k + 1) * HW])
            last = S_PER_CORE - 1
            emit_layer(last, 3, range(0, 1), lambda k: q_tiles[last][2][:, k * HW : (k + 1) * HW])
            emit_fc5(last)

    _split_ctrl_multiwaits(nc)
    return nc


_NC_CACHE = None


def _get_nc():
    global _NC_CACHE
    if _NC_CACHE is None:
        _NC_CACHE = _build_nc()
    return _NC_CACHE


def _to_e3m4(a, scale):
    return np.clip(a * scale, -14.0, 14.0).astype(ml_dtypes.float8_e3m4)


def _prep_core(inputs, c):
    """Build the per-core input map (numpy only, host-side layout prep)."""
    sl = slice(c * S_PER_CORE, (c + 1) * S_PER_CORE)

    def wimg(li):
        cin, cout = LAYERS[li]
        w = inputs[f"target_fc{li + 1}w"][sl, :, :, 0, 0]  # [S, Cout, Cin]
        # -> [S, 128, (Cin/128)*Cout] with img[s, p, k*Cout+co] = w[s, co, k*128+p]
        wt = w.transpose(0, 2, 1).reshape(S_PER_CORE, cin // 128, 128, cout)
        return wt.transpose(0, 2, 1, 3).reshape(S_PER_CORE, 128, -1)

    w1 = wimg(0)  # [S, 128, 16384] k-major: col = k*1024 + m*128
    # fc1 -> m-major: col = m*2048 + k*128
    w1 = (
        w1.reshape(S_PER_CORE, 128, 16, 8, 128)
        .transpose(0, 1, 3, 2, 4)
        .reshape(S_PER_CORE, 128, A_COLS)
    )
    wslab = np.ascontiguousarray(
        _to_e3m4(np.concatenate([w1] + [wimg(li) for li in (1, 2, 3)], axis=2), W_SCALE_FP8)
    ).view(np.uint8).reshape(S_PER_CORE, 128, -1).view(np.float16)

    x = inputs["target_in_vec"][sl].reshape(S_PER_CORE, 2048 // 128, 128, HW)
    ximg = x.transpose(2, 0, 1, 3).reshape(128, S_PER_CORE * X_COLS)
    ximg = np.ascontiguousarray(_to_e3m4(ximg, X_SCALE_FP8)).view(np.uint8).view(np.float16)

    w5 = inputs["target_fc5w"][sl, 0, :, 0, 0].astype(np.float16)  # [S, 128]
    w5img = np.zeros((128, S_PER_CORE, W5_COLS), np.float16)
    w5img[:, :, 0] = w5.T
    w5img = np.ascontiguousarray(w5img.reshape(128, -1))

    bias = np.zeros((S_PER_CORE, 128, BIAS_COLS), np.float32)
    for li, (cin, cout) in enumerate(LAYERS):
        b = inputs[f"target_fc{li + 1}b"][sl]  # [S, Cout]
        bias[:, :, BIAS_COL0[li] : BIAS_COL0[li] + cout // 128] = b.reshape(
            S_PER_CORE, cout // 128, 128
        ).transpose(0, 2, 1)
    bias[:, 0, 15] = inputs["target_fc5b"][sl, 0]
    bias = np.ascontiguousarray(bias.transpose(1, 0, 2).reshape(128, -1))

    return {"wslab": wslab, "ximg": ximg, "w5img": w5img, "bias": bias}


def kernel(**inputs):
    inputs = {k: np.asarray(v) for k, v in inputs.items()}
    nc = _get_nc()
    in_maps = [_prep_core(inputs, c) for c in range(N_CORES)]
    res = run_bass_kernel_spmd(nc, in_maps, list(range(N_CORES)))
    out = np.concatenate([np.asarray(res.results[c]["out"]) for c in range(N_CORES)], axis=0)
    return out.reshape(B, 8, 8).astype(np.float32)
